# revision 70
# baseline (speedup 1.0000x reference)
"""Bahdanau additive attention on 8 Trainium2 NeuronCores (Bass/Tile).

Problem (hardcoded): B=4, Q=256, K=1024, H=128, fp32.
  q_proj = query @ Wq.T ; k_proj = key @ Wk.T
  score[b,q,k] = sum_h v[h] * tanh(q_proj[b,q,h] + k_proj[b,k,h])
  attn = softmax(score, -1) ; ctx = attn @ value
The mask term is +(-1e-9)*(1-mask): below fp32 resolution of the O(1)
scores (measured max rel impact 3.5e-7 on attn), so it is skipped.

Sharding: data-parallel over the B*Q = 1024 query rows; core i handles
batch b = i//2, query rows (i%2)*128 ... +128, with that batch's full
key/value. No collectives.

Per-core algorithm (fully fused on-chip; the (Q,K,H) feat tensor never
touches HBM):
  - ~3us of dummy matmuls at t=0 warm the PE HAM clock-gate while the
    input DMAs stream in; a dummy activation pre-loads the tanh/exp
    function table.
  - kpT[h,k] = Wk @ key_b.T and qpT[h,q] = Wq @ q_slice.T via PE
    transposes + fp32 matmuls, pipelined at key-half granularity.
  - VectorE builds prefeat[:, q] = kpT + qpT[:,q] (per-partition
    tensor_scalar add, fp32 2x mode); ScalarE then runs tanh over
    multi-query chunks (amortizing the ~450-cycle per-instruction
    SBUF-access overhead), emitting feat in float32r. The first and
    last two queries run at half-K granularity to shorten the pipeline
    fill and drain.
  - score rows land in per-q PSUM partitions via a one-hot stationary:
    v embedded at column q of a 255-wide zero tile; accumulating
    matmuls add v.feat into PSUM partition q and +0 elsewhere
    (fp32r moving = 1 cycle/row, ~1.6e-4 rel err; exact-fp32 knob
    available via SCORE_DT).
  - softmax over k on the (128q x 1024k) PSUM score tiles: exp with
    fused per-partition accumulation (denominators), reciprocal,
    per-partition scale. No max-subtraction: |score| <= sum|v_h| ~ 9.4,
    exp cannot overflow.
  - context: PE-transpose unnormalized exp scores 128x128 blocks ->
    stationary, 8 accumulating fp32 matmuls against value chunks; the
    softmax normalization folds into the final PSUM->SBUF scale.
"""

import numpy as np

try:
    import concourse.bacc as bacc
except ImportError:  # fallback if the axon sitecustomize path isn't present
    import sys

    for _p in ("/root/.axon_site/_ro/trn_rl_repo", "/opt/trn_rl_repo"):
        if _p not in sys.path:
            sys.path.append(_p)
    import concourse.bacc as bacc
import concourse.tile as tile
from concourse import mybir
from concourse.bass_utils import run_bass_kernel_spmd
from concourse.masks import make_identity

F32 = mybir.dt.float32
F32R = mybir.dt.float32r
AF = mybir.ActivationFunctionType

B, Q, K, H = 4, 256, 1024, 128
N_CORES = 8
QPC = (B * Q) // N_CORES  # query rows per core = 128
KB = K // 128  # 8 key chunks of 128

# score-matmul moving dtype: F32R (TF32-like, 1 cyc/row) or F32 (exact,
# 4 cyc/row but PE becomes the bottleneck).
SCORE_DT = F32R

# tanh chunk schedule: small chunks first so ScalarE starts early, then
# steady-state 6 queries per instruction (big enough to amortize the
# per-instruction overhead, small enough that the PE's inter-burst idle
# stays under the ~3.4us HAM re-throttle window). The first two queries
# are handled separately (half-k pipeline startup).
CHUNKS = [2, 4] + [6] * 19 + [4]
assert sum(CHUNKS) == QPC - 4


def build_nc():
    nc = bacc.Bacc(None, target_bir_lowering=False)

    d_query = nc.declare_dram_parameter("q_slice", [QPC, H], F32, isOutput=False)
    d_key = nc.declare_dram_parameter("key_b", [K, H], F32, isOutput=False)
    d_value = nc.declare_dram_parameter("value_b", [K, H], F32, isOutput=False)
    d_wq = nc.declare_dram_parameter("Wq", [H, H], F32, isOutput=False)
    d_wk = nc.declare_dram_parameter("Wk", [H, H], F32, isOutput=False)
    d_v = nc.declare_dram_parameter("v", [H, 1], F32, isOutput=False)
    d_attn = nc.declare_dram_parameter("attn_out", [QPC, K], F32, isOutput=True)
    d_ctx = nc.declare_dram_parameter("ctx_out", [QPC, H], F32, isOutput=True)

    key_r = d_key.rearrange("(kb p) d -> p kb d", p=128)

    with tile.TileContext(nc) as tc:
        with (
            tc.tile_pool(name="const", bufs=1) as const,
            tc.tile_pool(name="head", bufs=1) as head,
            tc.tile_pool(name="pref", bufs=3) as prefp,
            tc.tile_pool(name="feat", bufs=4) as featp,
            tc.tile_pool(name="tail", bufs=1) as tailp,
            tc.tile_pool(name="tposb", bufs=4) as tposb,
            tc.tile_pool(name="pst", bufs=3, space="PSUM") as pst,
            tc.tile_pool(name="pskp", bufs=1, space="PSUM") as pskp,
            tc.tile_pool(name="pssc", bufs=1, space="PSUM") as pssc,
            tc.tile_pool(name="psctx", bufs=1, space="PSUM") as psctx,
        ):
            ident = const.tile([128, 128], F32)
            make_identity(nc, ident)

            # warm the ACT function-table (exp_and_others: tanh+exp) at
            # t=0 so the ~2.7us table DMA overlaps the input DMAs.
            warm = const.tile([128, 1], F32)
            nc.vector.memset(warm, 0.0)
            nc.scalar.activation(out=warm, in_=warm, func=AF.Tanh)

            # PE warm-up: ~3us of back-to-back dummy matmuls at t=0 so
            # the HAM clock-gate reaches full speed before the real
            # transposes/projections arrive. Targets ctx_ps, which the
            # first context matmul later overwrites (start=True).
            scratch = const.tile([128, 64], F32)
            nc.vector.memset(scratch, 0.0)
            ctx_ps = psctx.tile([QPC, H], F32)
            for _ in range(14):
                nc.tensor.matmul(
                    ctx_ps[0:64, 0:64],
                    lhsT=scratch,
                    rhs=scratch[:, 0:64],
                    start=True,
                    stop=True,
                )

            # ---- loads (small tensors first; key halves so transposes
            # can start at the half-way point; value last) ----
            wq_sb = head.tile([H, H], F32)
            wk_sb = head.tile([H, H], F32)
            qs_sb = head.tile([QPC, H], F32)
            v_sb = head.tile([H, 1], F32)
            key_sb = head.tile([128, KB, H], F32)
            val_sb = head.tile([128, KB, H], F32)
            # One HWDGE ring issues serially (~650ns each): order by when
            # each tensor gates the critical path — first key half, then
            # the small projection tensors, second key half, value last.
            nc.sync.dma_start(out=key_sb[:, 0:4, :], in_=key_r[:, 0:4, :])
            nc.sync.dma_start(out=wk_sb, in_=d_wk[:, :])
            nc.sync.dma_start(out=wq_sb, in_=d_wq[:, :])
            nc.sync.dma_start(out=qs_sb, in_=d_query[:, :])
            nc.sync.dma_start(out=key_sb[:, 4:8, :], in_=key_r[:, 4:8, :])
            nc.sync.dma_start(out=v_sb, in_=d_v[:, :])
            nc.sync.dma_start(
                out=val_sb, in_=d_value.rearrange("(kb p) d -> p kb d", p=128)
            )

            # ---- transposes (PE identity trick) + projections, ordered
            # by DMA arrival (engines execute in program order) ----
            def pe_transpose(dst_sb, src_sb):
                ps = pst.tile([128, 128], F32, tag="tps")
                nc.tensor.transpose(ps, src_sb, ident)
                nc.vector.tensor_copy(dst_sb, ps)

            kpT_ps = pskp.tile([H, K], F32)
            kpT = head.tile([H, K], F32)
            keyT = head.tile([H, K], F32)
            wqT = head.tile([H, H], F32)
            wkT = head.tile([H, H], F32)
            qsT = head.tile([H, QPC], F32)

            # first key half -> keyT -> kpT bank 0 -> SBUF copy (ScalarE)
            for kb in range(4):
                pe_transpose(keyT[:, 128 * kb : 128 * (kb + 1)], key_sb[:, kb, :])
            pe_transpose(wkT, wk_sb)
            for kb in range(4):
                nc.tensor.matmul(
                    kpT_ps[:, 128 * kb : 128 * (kb + 1)],
                    lhsT=wkT,
                    rhs=keyT[:, 128 * kb : 128 * (kb + 1)],
                    start=True,
                    stop=True,
                )
            nc.scalar.copy(kpT[:, 0:512], kpT_ps[:, 0:512])

            # qpT[h, q] = sum_d Wq[h, d] * query[q, d]
            pe_transpose(wqT, wq_sb)
            pe_transpose(qsT, qs_sb)
            qpT_ps = pst.tile([H, QPC], F32, tag="tps")
            nc.tensor.matmul(qpT_ps, lhsT=wqT, rhs=qsT, start=True, stop=True)
            qpT = head.tile([H, QPC], F32)
            nc.vector.tensor_copy(qpT, qpT_ps)

            # second key half -> kpT bank 1
            for kb in range(4, KB):
                pe_transpose(keyT[:, 128 * kb : 128 * (kb + 1)], key_sb[:, kb, :])
            for kb in range(4, KB):
                nc.tensor.matmul(
                    kpT_ps[:, 128 * kb : 128 * (kb + 1)],
                    lhsT=wkT,
                    rhs=keyT[:, 128 * kb : 128 * (kb + 1)],
                    start=True,
                    stop=True,
                )
            nc.scalar.copy(kpT[:, 512:1024], kpT_ps[:, 512:1024])

            # v embedded at column 127 of a 255-wide zero tile, in SCORE_DT.
            # lhsT slice [127-q : 255-q] puts v at stationary column q, so
            # matmul accumulation adds v.feat into PSUM partition q and +0
            # into every other partition (fp32r only supports tile_position
            # (0,0), so the stationary must span all 128 columns).
            # (VectorE program order: placed after the keyT/qpT copies it
            # would otherwise block; v lands by ~4.5us.)
            vz = const.tile([128, 255], F32)
            nc.vector.memset(vz, 0.0)
            vemb = const.tile([128, 255], SCORE_DT)
            nc.vector.tensor_copy(vemb, vz)
            nc.vector.tensor_copy(vemb[:, 127:128], v_sb)

            # ---- main loop: VectorE add -> ScalarE chunked tanh -> PE ----
            score0 = pssc.tile([128, 512], F32)
            score1 = pssc.tile([128, 512], F32)
            q = 0
            # First two queries flow in k-halves gated on the two kpT PSUM
            # banks, so the pipeline starts as soon as half the projection
            # is done instead of waiting for all of kpT.
            for q in range(2):
                prefeat = prefp.tile([H, max(CHUNKS) * K], F32, tag="pref")
                feat = featp.tile([H, max(CHUNKS) * K], SCORE_DT, tag="feat")
                for half, (sc, lo) in enumerate(((score0, 0), (score1, 512))):
                    nc.vector.tensor_scalar_add(
                        prefeat[:, lo : lo + 512],
                        kpT[:, lo : lo + 512],
                        qpT[:, q : q + 1],
                    )
                    nc.scalar.activation(
                        out=feat[:, lo : lo + 512],
                        in_=prefeat[:, lo : lo + 512],
                        func=AF.Tanh,
                    )
                    nc.tensor.matmul(
                        sc,
                        lhsT=vemb[:, 127 - q : 255 - q],
                        rhs=feat[:, lo : lo + 512],
                        start=(q == 0),
                        stop=False,
                    )
            q = 2
            for ci, csz in enumerate(CHUNKS):
                prefeat = prefp.tile([H, max(CHUNKS) * K], F32, tag="pref")
                feat = featp.tile([H, max(CHUNKS) * K], SCORE_DT, tag="feat")
                for i in range(csz):
                    nc.vector.tensor_scalar_add(
                        prefeat[:, K * i : K * (i + 1)],
                        kpT,
                        qpT[:, q + i : q + i + 1],
                    )
                nc.scalar.activation(
                    out=feat[:, 0 : K * csz],
                    in_=prefeat[:, 0 : K * csz],
                    func=AF.Tanh,
                )
                for i in range(csz):
                    st, sp = False, False
                    nc.tensor.matmul(
                        score0,
                        lhsT=vemb[:, 127 - (q + i) : 255 - (q + i)],
                        rhs=feat[:, K * i : K * i + 512],
                        start=st,
                        stop=sp,
                    )
                    nc.tensor.matmul(
                        score1,
                        lhsT=vemb[:, 127 - (q + i) : 255 - (q + i)],
                        rhs=feat[:, K * i + 512 : K * i + 1024],
                        start=st,
                        stop=sp,
                    )
                q += csz

            # ---- tail peel + softmax + context, interleaved so the
            # PE starts the context transposes between the two peel
            # halves and ScalarE never blocks on VectorE copies ----
            prefL = prefp.tile([H, max(CHUNKS) * K], F32, tag="pref")
            featL = featp.tile([H, max(CHUNKS) * K], SCORE_DT, tag="feat")
            for i in range(2):
                nc.vector.tensor_scalar_add(
                    prefL[:, K * i : K * (i + 1)],
                    kpT,
                    qpT[:, QPC - 2 + i : QPC - 1 + i],
                )

            def peel_half(lo, sc):
                for i in range(2):
                    qq = QPC - 2 + i
                    off = K * i + lo
                    nc.scalar.activation(
                        out=featL[:, off : off + 512],
                        in_=prefL[:, off : off + 512],
                        func=AF.Tanh,
                    )
                    nc.tensor.matmul(
                        sc,
                        lhsT=vemb[:, 127 - qq : 255 - qq],
                        rhs=featL[:, off : off + 512],
                        start=False,
                        stop=(i == 1),
                    )

            attn_un0 = tailp.tile([128, 512], F32)
            attn_un1 = tailp.tile([128, 512], F32)
            den0 = tailp.tile([128, 1], F32)
            den1 = tailp.tile([128, 1], F32)

            peel_half(0, score0)
            nc.scalar.activation(out=attn_un0, in_=score0, func=AF.Exp, accum_out=den0)

            # transposes of the first four 128-col blocks run on PE while
            # ScalarE does the half-1 tanhs and exp1
            attnT = [None] * KB
            for kb in range(4):
                tps = pst.tile([128, 128], F32, tag="tps")
                nc.tensor.transpose(tps, attn_un0[:, 128 * kb : 128 * (kb + 1)], ident)
                attnT[kb] = tposb.tile([128, 128], F32, tag="attnT", name=f"attnT{kb}")
                nc.vector.tensor_copy(attnT[kb], tps)

            peel_half(512, score1)
            nc.scalar.activation(out=attn_un1, in_=score1, func=AF.Exp, accum_out=den1)

            den = tailp.tile([128, 1], F32)
            rden = tailp.tile([128, 1], F32)
            nc.vector.tensor_add(den, den0, den1)
            nc.vector.reciprocal(rden, den)
            attn = tailp.tile([128, K], F32)
            nc.vector.tensor_scalar_mul(attn[:, 0:512], attn_un0, rden)
            nc.vector.tensor_scalar_mul(attn[:, 512:1024], attn_un1, rden)
            nc.sync.dma_start(out=d_attn[:, :], in_=attn)

            for kb in range(KB):
                if kb >= 4:
                    tps = pst.tile([128, 128], F32, tag="tps")
                    nc.tensor.transpose(
                        tps, attn_un1[:, 128 * (kb - 4) : 128 * (kb - 3)], ident
                    )
                    attnT[kb] = tposb.tile([128, 128], F32, tag="attnT", name=f"attnT{kb}")
                    nc.scalar.copy(attnT[kb], tps)
                nc.tensor.matmul(
                    ctx_ps,
                    lhsT=attnT[kb],
                    rhs=val_sb[:, kb, :],
                    start=(kb == 0),
                    stop=(kb == KB - 1),
                )
            ctx_sb = tailp.tile([QPC, H], F32)
            nc.vector.tensor_scalar_mul(ctx_sb, ctx_ps, rden)
            nc.sync.dma_start(out=d_ctx[:, :], in_=ctx_sb)

    nc.finalize()
    return nc


_NC = None


def _get_nc():
    global _NC
    if _NC is None:
        _NC = build_nc()
    return _NC


def _shard(inputs):
    query = np.ascontiguousarray(np.asarray(inputs["query"], dtype=np.float32))
    key = np.ascontiguousarray(np.asarray(inputs["key"], dtype=np.float32))
    value = np.ascontiguousarray(np.asarray(inputs["value"], dtype=np.float32))
    Wq = np.ascontiguousarray(np.asarray(inputs["Wq"], dtype=np.float32))
    Wk = np.ascontiguousarray(np.asarray(inputs["Wk"], dtype=np.float32))
    v = np.ascontiguousarray(np.asarray(inputs["v"], dtype=np.float32).reshape(H, 1))
    in_maps = []
    for i in range(N_CORES):
        b, qh = divmod(i, 2)
        qs = qh * QPC
        in_maps.append(
            {
                "q_slice": np.ascontiguousarray(query[b, qs : qs + QPC, :]),
                "key_b": key[b],
                "value_b": value[b],
                "Wq": Wq,
                "Wk": Wk,
                "v": v,
            }
        )
    return in_maps


def _assemble(results):
    attn = np.empty((B, Q, K), dtype=np.float32)
    ctx = np.empty((B, Q, H), dtype=np.float32)
    for i in range(N_CORES):
        b, qh = divmod(i, 2)
        qs = qh * QPC
        attn[b, qs : qs + QPC, :] = results[i]["attn_out"]
        ctx[b, qs : qs + QPC, :] = results[i]["ctx_out"]
    return attn, ctx


def run(trace=False, **inputs):
    nc = _get_nc()
    res = run_bass_kernel_spmd(nc, _shard(inputs), list(range(N_CORES)), trace=trace)
    return _assemble(res.results), res


def kernel(**inputs):
    (attn, ctx), _ = run(trace=False, **inputs)
    return attn, ctx
